# revision 18
# baseline (speedup 1.0000x reference)
"""Trainium2 Bass kernel for nn_MultiHeadAttention_781684048588.

MHA + FiLM stylization block. Data-parallel over batch: core b handles batch
element b (B == n_cores == 8). No collectives.

Per-core pipeline (all f32):
  A. Load q,k,v natural; build qT,kT,vT via PE matmul-transposes (lhsT.T @ I);
     project QT = WqT@qT, KT = WkT@kT (feature-major) and V (natural, padded
     with a ones column per head for softmax sums).
  B. Per head pair (dk=64 -> two heads share the 128-row PE array):
     pass-T: scoresT[k,q] on PE -> ACT exp (raw) -> attnV matmul
             (Vpad stationary) giving oT[65,q]; row 64 = softmax sums s^T.
             o normalized via rank-1 (ones x 1/s) broadcast + DVE mul.
     s-plumb: ln(s)^T -> tiny PE transposes -> -ln(s) per-partition [128,h].
     pass-N: scores[q,k] on PE -> ACT exp(x/8 - ln s) = normalized attn
             directly -> DMA out (the 256 MB attn output).
  C. Epilogue: fc (OT is already the lhsT), LN, FiLM from latent, SiLU,
     Ws2 (via PE transpose of silu(h)), residual, LN -> out.
"""

import os
import numpy as np
from contextlib import ExitStack

B, L, D = 8, 1024, 512
H, DK, DV = 8, 64, 64
DLAT = 512
P = 128
NQ = L // P    # 8 q/k position tiles
NC = D // P    # 4 feature chunks
NH2 = H // 2   # head pairs

_CACHE = {}


def _split_multiwait_bir(bir_json: bytes) -> bytes:
    """Walrus in this toolchain rejects instructions carrying more than one
    semaphore wait. Hoist extra waits into standalone single-wait
    EventSemaphore instructions placed immediately before the instruction on
    the same engine (semantically identical: waits run in program order)."""
    import orjson

    m = orjson.loads(bir_json)
    n = 0
    for fn in m.get("functions", []):
        for blk in fn.get("blocks", []):
            new_insts = []
            for inst in blk.get("instructions", []):
                si = inst.get("sync_info")
                waits = (si or {}).get("on_wait") or []
                if len(waits) > 1 and inst.get("engine"):
                    for w in waits[:-1]:
                        n += 1
                        new_insts.append({
                            "debug": inst.get("debug", 0),
                            "engine": inst["engine"],
                            "ins": [], "outs": [],
                            "name": f"antw-{n}",
                            "opcode": "EventSemaphore",
                            "sync_info": {"on_update": [], "on_wait": [w]},
                        })
                    si["on_wait"] = [waits[-1]]
                new_insts.append(inst)
            blk["instructions"] = new_insts
    return orjson.dumps(m)


def _install_bir_patch():
    import concourse.bass_utils as bu

    if getattr(bu.compile_bir_kernel, "_antw_patched", False):
        return
    orig = bu.compile_bir_kernel

    def patched(bir_json, tmpdir, neff_name="file.neff"):
        return orig(_split_multiwait_bir(bytes(bir_json)), tmpdir, neff_name)

    patched._antw_patched = True
    bu.compile_bir_kernel = patched
    import concourse.bass2jax as b2j

    b2j.compile_bir_kernel = patched


def build_bass():
    import concourse.bass as bass
    import concourse.mybir as mybir
    import concourse.tile as tile
    from concourse.bass import ts, ds
    from concourse.masks import make_identity

    F = mybir.dt.float32
    AF = mybir.ActivationFunctionType
    ALU = mybir.AluOpType

    nc = bass.Bass()

    # ---- DRAM I/O (per-core shapes) ----
    q_d = nc.dram_tensor("q", [L, D], F, kind="ExternalInput")
    k_d = nc.dram_tensor("k", [L, D], F, kind="ExternalInput")
    v_d = nc.dram_tensor("v", [L, D], F, kind="ExternalInput")
    lat_d = nc.dram_tensor("latent", [1, DLAT], F, kind="ExternalInput")
    Wq_d = nc.dram_tensor("Wq", [D, H * DK], F, kind="ExternalInput")
    bq_d = nc.dram_tensor("bq", [H * DK], F, kind="ExternalInput")
    Wk_d = nc.dram_tensor("Wk", [D, H * DK], F, kind="ExternalInput")
    bk_d = nc.dram_tensor("bk", [H * DK], F, kind="ExternalInput")
    Wv_d = nc.dram_tensor("Wv", [D, H * DV], F, kind="ExternalInput")
    bv_d = nc.dram_tensor("bv", [H * DV], F, kind="ExternalInput")
    Wfc_d = nc.dram_tensor("Wfc", [H * DV, D], F, kind="ExternalInput")
    bfc_d = nc.dram_tensor("bfc", [D], F, kind="ExternalInput")
    Ws1_d = nc.dram_tensor("Ws1", [DLAT, 2 * D], F, kind="ExternalInput")
    bs1_d = nc.dram_tensor("bs1", [2 * D], F, kind="ExternalInput")
    Ws2_d = nc.dram_tensor("Ws2", [D, D], F, kind="ExternalInput")
    bs2_d = nc.dram_tensor("bs2", [D], F, kind="ExternalInput")
    eng_d = nc.dram_tensor("en_g", [D], F, kind="ExternalInput")
    enb_d = nc.dram_tensor("en_b", [D], F, kind="ExternalInput")
    lng_d = nc.dram_tensor("ln_g", [D], F, kind="ExternalInput")
    lnb_d = nc.dram_tensor("ln_b", [D], F, kind="ExternalInput")
    out_d = nc.dram_tensor("out", [L, D], F, kind="ExternalOutput")
    attn_d = nc.dram_tensor("attn", [H, L, L], F, kind="ExternalOutput")

    def row(dram1d, n):
        # 1-D DRAM tensor as a [1, n] AP
        return dram1d[:].rearrange("(one n) -> one n", one=1)

    def col_ap(dram1d):
        # 1-D DRAM tensor [D] as [128, NC] partition-major AP
        return dram1d[:].rearrange("(c p) -> p c", p=P)

    with tile.TileContext(nc) as tc, ExitStack() as ctx:
        const = ctx.enter_context(tc.tile_pool(name="const", bufs=1))
        persist = ctx.enter_context(tc.tile_pool(name="persist", bufs=1))
        small = ctx.enter_context(tc.tile_pool(name="small", bufs=4))
        ps = ctx.enter_context(tc.tile_pool(name="ps", bufs=2, space="PSUM"))
        ps_o = ctx.enter_context(tc.tile_pool(name="ps_o", bufs=2, space="PSUM"))

        ident = const.tile([P, P], F)
        make_identity(nc, ident)
        ones_row = const.tile([1, P], F)
        nc.vector.memset(ones_row, 1.0)
        eps1 = const.tile([P, 1], F)
        nc.vector.memset(eps1, 1e-5)
        eps2 = const.tile([P, 1], F)
        nc.vector.memset(eps2, 1e-6)

        # persistent SBUF tensors (live to the end)
        q_sb = persist.tile([P, NQ, D], F)     # natural q (residual + transpose src)
        OT = persist.tile([P, NC, L], F, tag="otT")  # o^T (normalized) -> fc lhsT
        nls = persist.tile([P, NQ, H], F)      # -ln(s) per q-partition, per head

        nc.sync.dma_start(out=q_sb, in_=q_d[:, :].rearrange("(t p) d -> p t d", p=P))

        # live through phases A+B only
        persB = ctx.enter_context(tc.tile_pool(name="persB", bufs=1))
        QT = persB.tile([P, NC, L], F)         # Q^T  [dk-feature, l]
        KT = persB.tile([P, NC, L], F)
        Vp = persB.tile([P, NQ, H * 65], F)    # V padded: per head 64 cols + ones
        bqT = persB.tile([P, NC], F, tag="bias", bufs=2)
        bkT = persB.tile([P, NC], F, tag="bias", bufs=2)
        nc.sync.dma_start(out=bqT, in_=col_ap(bq_d))
        nc.sync.dma_start(out=bkT, in_=col_ap(bk_d))

        # ---------------- Phase A: transposes + projections ----------------
        with tc.tile_pool(name="phA", bufs=1) as phA:
            bv_row = phA.tile([1, H * DV], F, tag="vr")
            nc.sync.dma_start(out=bv_row, in_=row(bv_d, H * DV))
            nc.vector.memset(Vp, 1.0)
            Vp_h = Vp[:, :, :].rearrange("p t (h c) -> p t h c", c=65)

            def transpose_in(x_sb, xT):
                # xT[:, dc, lb*128:...] = x[lb-tile, dc-chunk].T
                for dc in range(NC):
                    for lb in range(NQ):
                        pt = ps.tile([P, 1024], F, tag="ps", name=f"ptr_{dc}_{lb}")
                        nc.tensor.matmul(
                            pt[:, 0:P], x_sb[:, lb, ts(dc, P)], ident,
                            start=True, stop=True,
                        )
                        if (dc + lb) % 2 == 0:
                            nc.vector.tensor_copy(out=xT[:, dc, ts(lb, P)], in_=pt[:, 0:P])
                        else:
                            nc.scalar.copy(out=xT[:, dc, ts(lb, P)], in_=pt[:, 0:P])

            def project_T(xT, W_sb, bT, XT):
                # XT[mb, :] = sum_dc W[dc, mb].T @ xT[dc, :] + b  (feature-major)
                for mb in range(NC):
                    for nb in range(2):
                        pq = ps.tile([P, 1024], F, tag="ps", name=f"ppr_{mb}_{nb}")
                        for dc in range(NC):
                            nc.tensor.matmul(
                                pq[:, 0:512],
                                W_sb[:, dc, ts(mb, P)],
                                xT[:, dc, ds(nb * 512, 512)],
                                start=(dc == 0), stop=(dc == NC - 1),
                            )
                        nc.vector.tensor_scalar_add(
                            out=XT[:, mb, ds(nb * 512, 512)],
                            in0=pq[:, 0:512],
                            scalar1=bT[:, mb : mb + 1],
                        )

            # q -> qT -> QT
            qT = phA.tile([P, NC, L], F, tag="xt", bufs=2)
            Wq_sb = phA.tile([P, NC, H * DK], F, tag="w", bufs=2)
            nc.sync.dma_start(out=Wq_sb, in_=Wq_d[:, :].rearrange("(c p) n -> p c n", p=P))
            transpose_in(q_sb, qT)
            project_T(qT, Wq_sb, bqT, QT)

            # k -> kT -> KT
            k_sb = phA.tile([P, NQ, D], F, tag="xin")
            kT = phA.tile([P, NC, L], F, tag="xt", bufs=2)
            Wk_sb = phA.tile([P, NC, H * DK], F, tag="w", bufs=2)
            nc.sync.dma_start(out=k_sb, in_=k_d[:, :].rearrange("(t p) d -> p t d", p=P))
            nc.sync.dma_start(out=Wk_sb, in_=Wk_d[:, :].rearrange("(c p) n -> p c n", p=P))
            transpose_in(k_sb, kT)
            project_T(kT, Wk_sb, bkT, KT)

            # v -> vT -> V (padded layout, ones col for softmax sums)
            v_sb = phA.tile([P, NQ, D], F, tag="xin")
            vT = phA.tile([P, NC, L], F, tag="xt", bufs=2)
            Wv_sb = phA.tile([P, NC, H * DV], F, tag="w", bufs=2)
            nc.sync.dma_start(out=v_sb, in_=v_d[:, :].rearrange("(t p) d -> p t d", p=P))
            nc.sync.dma_start(out=Wv_sb, in_=Wv_d[:, :].rearrange("(c p) n -> p c n", p=P))
            transpose_in(v_sb, vT)
            for lb in range(NQ):
                pv = ps.tile([P, 1024], F, tag="ps", name=f"pv_{lb}")
                for dc in range(NC):
                    nc.tensor.matmul(
                        pv[:, 0:512],
                        vT[:, dc, ts(lb, P)],
                        Wv_sb[:, dc, :],
                        start=(dc == 0), stop=False,
                    )
                nc.tensor.matmul(
                    pv[:, 0:512], ones_row[0:1, 0:P], bv_row,
                    start=False, stop=True,
                )
                nc.vector.tensor_copy(
                    out=Vp_h[:, lb, :, 0:64],
                    in_=pv[:, 0:512].rearrange("p (h c) -> p h c", c=64),
                )

        # ---------------- Phase B: attention, per head pair ----------------
        with tc.tile_pool(name="eT", bufs=14) as eTp, \
             tc.tile_pool(name="attn_st", bufs=3) as atp:
            for hp in range(NH2):
                heads = (2 * hp, 2 * hp + 1)
                eTs = {}
                lns2 = small.tile([1, 2 * L], F, tag="lns", bufs=2,
                                  name=f"lns2_{hp}")
                # pass-T scores + exp (raw), row-tiled across the head pair
                for h in heads:
                    r0 = (h % 2) * 64
                    th = h // 2
                    eTs[h] = [
                        eTp.tile([P, L], F, tag="eT", name=f"eT_{h}_{kb}")
                        for kb in range(NQ)
                    ]
                    for kb in range(NQ):
                        pt = ps.tile([P, 1024], F, tag="ps")
                        for nb in range(2):
                            nc.tensor.matmul(
                                pt[:, ds(nb * 512, 512)],
                                KT[r0 : r0 + 64, th, ts(kb, P)],
                                QT[r0 : r0 + 64, th, ds(nb * 512, 512)],
                                start=True, stop=True,
                            )
                        nc.scalar.activation(
                            out=eTs[h][kb], in_=pt, func=AF.Exp, scale=0.125,
                        )
                # attnV: oT[65, L] accumulated over k chunks
                for h in heads:
                    po = ps_o.tile([65, L], F, tag="oT")
                    for kb in range(NQ):
                        for nb in range(2):
                            nc.tensor.matmul(
                                po[:, ds(nb * 512, 512)],
                                Vp[:, kb, ds(h * 65, 65)],
                                eTs[h][kb][:, ds(nb * 512, 512)],
                                start=(kb == 0), stop=(kb == NQ - 1),
                            )
                    # softmax sums (row 64): 1/s and ln(s) straight from PSUM
                    r_row = small.tile([1, L], F, tag="rr", bufs=2, name=f"r_row_{h}")
                    nc.vector.reciprocal(out=r_row, in_=po[64:65, :])
                    nc.scalar.activation(
                        out=lns2[0:1, ds((h % 2) * L, L)], in_=po[64:65, :],
                        func=AF.Ln,
                    )
                    pb = ps.tile([P, 1024], F, tag="ps")
                    for nb in range(2):
                        nc.tensor.matmul(
                            pb[0:64, ds(nb * 512, 512)],
                            ones_row[0:1, 0:64],
                            r_row[0:1, ds(nb * 512, 512)],
                            start=True, stop=True,
                        )
                    bc_sb = small.tile([64, L], F, tag="bc", bufs=2)
                    nc.scalar.copy(out=bc_sb, in_=pb[0:64, :])
                    r0 = (h % 2) * 64
                    nc.vector.tensor_tensor(
                        out=OT[r0 : r0 + 64, h // 2, :],
                        in0=po[0:64, :], in1=bc_sb, op=ALU.mult,
                    )

                # s-plumbing: -ln(s) transposed to per-q-partition layout
                for h in heads:
                    for qb in range(NQ):
                        pt = ps.tile([P, 1024], F, tag="ps",
                                     name=f"pt_s_{h}_{qb}")
                        nc.tensor.matmul(
                            pt[:, 0:1],
                            lns2[0:1, ds((h % 2) * L + qb * P, P)],
                            ident[0:1, 0:1],
                            start=True, stop=True,
                        )
                        nc.vector.tensor_scalar_mul(
                            out=nls[:, qb, h : h + 1],
                            in0=pt[:, 0:1], scalar1=-1.0,
                        )

                # pass-N: natural scores + normalized exp -> attn out
                for h in heads:
                    r0 = (h % 2) * 64
                    th = h // 2
                    for qb in range(NQ):
                        pn = ps.tile([P, 1024], F, tag="ps")
                        for nb in range(2):
                            nc.tensor.matmul(
                                pn[:, ds(nb * 512, 512)],
                                QT[r0 : r0 + 64, th, ts(qb, P)],
                                KT[r0 : r0 + 64, th, ds(nb * 512, 512)],
                                start=True, stop=True,
                            )
                        at = atp.tile([P, L], F, tag="at")
                        nc.scalar.activation(
                            out=at, in_=pn, func=AF.Exp, scale=0.125,
                            bias=nls[:, qb, h : h + 1],
                        )
                        nc.sync.dma_start(out=attn_d[h, ts(qb, P), :], in_=at)

        # ---------------- Phase C: epilogue ----------------
        with tc.tile_pool(name="phC", bufs=1) as phC, \
             tc.tile_pool(name="phC4", bufs=2) as phC4:
            Wfc_sb = phC.tile([P, NC, D], F)
            Ws1_sb = phC.tile([P, NC, 2 * D], F)
            Ws2_sb = phC.tile([P, NC, D], F)
            nc.sync.dma_start(out=Wfc_sb, in_=Wfc_d[:, :].rearrange("(c p) n -> p c n", p=P))
            nc.sync.dma_start(out=Ws1_sb, in_=Ws1_d[:, :].rearrange("(c p) n -> p c n", p=P))
            nc.sync.dma_start(out=Ws2_sb, in_=Ws2_d[:, :].rearrange("(c p) n -> p c n", p=P))
            rows = phC.tile([1, 8 * D], F, tag="rows")
            bfc_row = rows[0:1, ds(0, D)]
            bs1_row = rows[0:1, ds(D, 2 * D)]
            bs2_row = rows[0:1, ds(3 * D, D)]
            eng_row = rows[0:1, ds(4 * D, D)]
            enb_row = rows[0:1, ds(5 * D, D)]
            sc1 = rows[0:1, ds(6 * D, D)]       # 1 + scale
            G_row = rows[0:1, ds(7 * D, D)]
            lat_col = phC.tile([P, NC], F, tag="r5")
            nc.sync.dma_start(out=bfc_row, in_=row(bfc_d, D))
            nc.sync.dma_start(out=bs1_row, in_=row(bs1_d, 2 * D))
            nc.sync.dma_start(out=bs2_row, in_=row(bs2_d, D))
            nc.sync.dma_start(out=eng_row, in_=row(eng_d, D))
            nc.sync.dma_start(out=enb_row, in_=row(enb_d, D))
            nc.sync.dma_start(out=lat_col, in_=lat_d[0, :].rearrange("(c p) -> p c", p=P))
            # ln_g / ln_b broadcast straight from DRAM
            LG_bc = phC.tile([P, D], F, tag="g0")
            LB_bc = phC.tile([P, D], F, tag="g1")
            import concourse.bass as bass_mod
            lng_ap = lng_d[:]
            lnb_ap = lnb_d[:]
            nc.sync.dma_start(
                out=LG_bc,
                in_=bass_mod.AP(tensor=lng_ap.tensor, offset=lng_ap.offset,
                                ap=[[0, P], [1, D]]),
            )
            nc.sync.dma_start(
                out=LB_bc,
                in_=bass_mod.AP(tensor=lnb_ap.tensor, offset=lnb_ap.offset,
                                ap=[[0, P], [1, D]]),
            )

            # latent path: lat = silu(latent) @ Ws1 + bs1  -> scale/shift rows
            slat = phC.tile([P, NC], F, tag="r6")
            nc.scalar.activation(out=slat, in_=lat_col, func=AF.Sigmoid)
            nc.vector.tensor_tensor(out=slat, in0=slat, in1=lat_col, op=ALU.mult)
            pl = ps.tile([P, 1024], F, tag="ps")
            for nb in range(2):
                for cc in range(NC):
                    nc.tensor.matmul(
                        pl[0:1, ds(nb * 512, 512)],
                        slat[:, cc : cc + 1],
                        Ws1_sb[:, cc, ds(nb * 512, 512)],
                        start=(cc == 0), stop=False,
                    )
                nc.tensor.matmul(
                    pl[0:1, ds(nb * 512, 512)],
                    ones_row[0:1, 0:1],
                    bs1_row[0:1, ds(nb * 512, 512)],
                    start=False, stop=True,
                )
            S_row = phC.tile([1, D], F, tag="r9")
            nc.vector.tensor_scalar_add(out=sc1, in0=pl[0:1, 0:D], scalar1=1.0)
            nc.vector.tensor_tensor(out=G_row, in0=sc1, in1=eng_row, op=ALU.mult)
            nc.vector.tensor_tensor(out=S_row, in0=sc1, in1=enb_row, op=ALU.mult)
            nc.vector.tensor_tensor(out=S_row, in0=S_row, in1=pl[0:1, D : 2 * D], op=ALU.add)
            G_bc = phC.tile([P, D], F, tag="g2")
            S_bc = phC.tile([P, D], F, tag="g3")
            for (rw, bc) in ((G_row, G_bc), (S_row, S_bc)):
                pg = ps.tile([P, 1024], F, tag="ps")
                nc.tensor.matmul(pg[:, 0:D], ones_row[0:1, 0:P], rw, start=True, stop=True)
                nc.scalar.copy(out=bc, in_=pg[:, 0:D])

            # fc -> LN(en) -> FiLM -> SiLU
            sh_sb = phC.tile([P, NQ, D], F, tag="sh")
            for qb in range(NQ):
                pf = ps.tile([P, 1024], F, tag="ps")
                for cc in range(NC):
                    nc.tensor.matmul(
                        pf[:, 0:D], OT[:, cc, ts(qb, P)], Wfc_sb[:, cc, :],
                        start=(cc == 0), stop=False,
                    )
                nc.tensor.matmul(pf[:, 0:D], ones_row[0:1, 0:P], bfc_row,
                                 start=False, stop=True)
                stats = small.tile([P, 6], F, tag="st")
                mv = small.tile([P, 2], F, tag="mv")
                nc.vector.bn_stats(out=stats, in_=pf[:, 0:D])
                nc.vector.bn_aggr(out=mv, in_=stats)
                nc.scalar.activation(out=mv[:, 1:2], in_=mv[:, 1:2],
                                     func=AF.Sqrt, bias=eps1)
                nc.vector.reciprocal(out=mv[:, 1:2], in_=mv[:, 1:2])
                xn = small.tile([P, D], F, tag="xn", bufs=2)
                nc.vector.tensor_scalar(
                    out=xn, in0=pf[:, 0:D],
                    scalar1=mv[:, 0:1], scalar2=mv[:, 1:2],
                    op0=ALU.subtract, op1=ALU.mult,
                )
                nc.vector.tensor_tensor(out=xn, in0=xn, in1=G_bc, op=ALU.mult)
                nc.vector.tensor_tensor(out=xn, in0=xn, in1=S_bc, op=ALU.add)
                sg = small.tile([P, D], F, tag="sg", bufs=2, name=f"sg_{qb}")
                nc.scalar.activation(out=sg, in_=xn, func=AF.Sigmoid)
                nc.vector.tensor_tensor(out=sh_sb[:, qb, :], in0=xn, in1=sg,
                                        op=ALU.mult)

            # transpose silu(h) for the Ws2 matmul (recycles OT's slot)
            shT = persist.tile([P, NC, L], F, tag="otT")
            for cc in range(NC):
                for qb in range(NQ):
                    pt = ps.tile([P, 1024], F, tag="ps")
                    nc.tensor.matmul(
                        pt[:, 0:P], sh_sb[:, qb, ts(cc, P)], ident,
                        start=True, stop=True,
                    )
                    if (cc + qb) % 2 == 0:
                        nc.vector.tensor_copy(out=shT[:, cc, ts(qb, P)], in_=pt[:, 0:P])
                    else:
                        nc.scalar.copy(out=shT[:, cc, ts(qb, P)], in_=pt[:, 0:P])

            # h2 = silu(h) @ Ws2 + bs2; out = LN(h2 + q)
            for qb in range(NQ):
                ph2 = ps.tile([P, 1024], F, tag="ps")
                for cc in range(NC):
                    nc.tensor.matmul(
                        ph2[:, 0:D], shT[:, cc, ts(qb, P)], Ws2_sb[:, cc, :],
                        start=(cc == 0), stop=False,
                    )
                nc.tensor.matmul(ph2[:, 0:D], ones_row[0:1, 0:P], bs2_row,
                                 start=False, stop=True)
                hr = phC4.tile([P, D], F, tag="hr")
                nc.vector.tensor_tensor(out=hr, in0=ph2[:, 0:D], in1=q_sb[:, qb, :],
                                        op=ALU.add)
                stats = small.tile([P, 6], F, tag="st")
                mv = small.tile([P, 2], F, tag="mv")
                nc.vector.bn_stats(out=stats, in_=hr)
                nc.vector.bn_aggr(out=mv, in_=stats)
                nc.scalar.activation(out=mv[:, 1:2], in_=mv[:, 1:2],
                                     func=AF.Sqrt, bias=eps2)
                nc.vector.reciprocal(out=mv[:, 1:2], in_=mv[:, 1:2])
                ob = phC4.tile([P, D], F, tag="ob")
                nc.vector.tensor_scalar(
                    out=ob, in0=hr,
                    scalar1=mv[:, 0:1], scalar2=mv[:, 1:2],
                    op0=ALU.subtract, op1=ALU.mult,
                )
                nc.vector.tensor_tensor(out=ob, in0=ob, in1=LG_bc, op=ALU.mult)
                nc.vector.tensor_tensor(out=ob, in0=ob, in1=LB_bc, op=ALU.add)
                nc.sync.dma_start(out=out_d[ts(qb, P), :], in_=ob)

    return nc


def _get_nc():
    if "nc" not in _CACHE:
        _CACHE["nc"] = build_bass()
    return _CACHE["nc"]


def kernel(**inputs):
    _install_bir_patch()
    from concourse.bass_utils import run_bass_kernel_spmd

    nc = _get_nc()
    names_per_batch = ("q", "k", "v", "latent")
    in_maps = []
    for b in range(B):
        m = {}
        for name, arr in inputs.items():
            a = np.ascontiguousarray(np.asarray(arr), dtype=np.float32)
            if name in names_per_batch:
                m[name] = np.ascontiguousarray(a[b])
            else:
                m[name] = a
        in_maps.append(m)

    res = run_bass_kernel_spmd(nc, in_maps, core_ids=list(range(B)))
    out = np.stack([r["out"] for r in res.results])
    attn = np.stack([r["attn"] for r in res.results])
    return out, attn
